# revision 19
# baseline (speedup 1.0000x reference)
"""FBPinn forward kernel for Trainium2 (8 NeuronCores, Bass/Tile).

The module computes y(x) = tanh(x) * sum_w [win_w(x)>1e-3] * win_w(x) * MLP_w(x)
for 1M scalar points x in [0,100) -- a fixed 1D function of x. Tolerance is
rel 2e-2, so a piecewise-linear table on a coarse grid suffices (measured
~2e-3 absmax with 128 cells/core, dominated by interpolating through the
win>1e-3 mask jumps, which this kernel does NOT special-case).

Per core (12.5-wide domain slice, 128 cells, one cell per SBUF partition):
  1. phase B: evaluate the function at the 129 knots of a uniform grid using
     the <=12 active per-window MLPs (block-diagonal-packed fp16 PE matmuls,
     fp32 PSUM accumulate). All activations are Tanh -- window sigmoids use
     sigmoid(z)=(1+tanh(z/2))/2 -- so the ACT table never swaps; a dummy
     activation at t=0 prefetches it. Knot x values are generated on-chip
     (f32 iota) and folded into activation scale/bias host-side.
  2. phase C: the final sum-over-slots matmul is done twice with lhsT=term
     shifted by one knot, yielding knot values directly in partition-major
     [128,1] columns; cell records (ulo, dv) take 2 fused DVE ops.
  3. phase D: points are packed (host side) so partition p holds exactly the
     points of cell p (S slots). The host sends t=(x-cell_left)/h as fp16;
     interpolation is ONE fused tensor_scalar per chunk: y = t*dv_p + ulo_p,
     written back as fp16 (host upcasts).
DMA is 2 input issues (one f32 const pack; one f16 stream carrying w2 then
the points) + 4 output stores. Host shards points by domain across the
8 cores, packs slots, and un-permutes the outputs.
"""

import numpy as np

# ---------------- problem constants (hardcoded from the module spec) ----------
NW = 30
DOM0, DOM1 = 0.0, 100.0
OVERLAP = 0.25
NEURONS = 32
THRESH = 0.001
N = 1_000_000

NCORES = 8
P = 128                      # SBUF partitions
C_LOC = P                    # cells per core: one per partition
DW = 12.5                    # per-core domain width
H = DW / C_LOC               # cell width = 0.09765625 (exact in fp32)
INVH = C_LOC / DW
NG = 3                       # window groups of 4 per core
NSLOT = 4 * NG               # window slots per core
NK = C_LOC + 1               # knots per core (129)
NKP = 132                    # padded iota width
NCHO = 4                     # phase-D output chunks
S_DEFAULT = 1104             # point slots per cell (max bin count 1084 + pad)
W2C = P * NG                 # fp16 w2 columns in the tw stream

# f32 const-pack A [P, CWA]: per-partition data (h1-critical, small rows)
O_SC1 = 0
O_BI1 = O_SC1 + NG
O_B2 = O_BI1 + NG
O_W3 = O_B2 + NG
O_XL = O_W3 + NSLOT * NG
O_XR = O_XL + 1
CWA = O_XR + 1
# f32 const-pack B [44, CWB]: window-slot data (rows 0:12 and 32:44 banks)
O_S12 = 0
O_B12 = O_S12 + 1
O_WM = O_B12 + 1             # [NSLOT, NK] window mask (0.25*mask), dual bank
O_B3 = O_WM + NK
O_ON = O_B3 + 1
CWB = O_ON + 1
PB = 32 + NSLOT              # partition rows of pack B


# ---------------- geometry (host, input-independent) --------------------------
def _partition_geom():
    width = (DOM1 - DOM0) / NW
    sub = np.zeros((NW, 2), np.float32)
    for i in range(NW):
        sub[i, 0] = DOM0 if i == 0 else DOM0 + (i - OVERLAP / 2) * width
        sub[i, 1] = DOM1 if i == NW - 1 else DOM0 + (i + 1 + OVERLAP / 2) * width
    means = (sub[:, 0] + sub[:, 1]) / 2
    std = (sub[:, 1] - sub[:, 0]) / 2
    mid = np.zeros(NW + 1, np.float32)
    mid[0] = sub[0, 0]
    mid[-1] = sub[-1, 1]
    for i in range(1, NW):
        mid[i] = (sub[i - 1, 1] + sub[i, 0]) / 2
    return means.astype(np.float32), std.astype(np.float32), mid.astype(np.float32)


def _win64(l, r, x):
    return 1.0 / (1 + np.exp(-(x - l))) / (1 + np.exp(x - r))


def _bisect64(l, r, lo, hi, rising):
    for _ in range(200):
        m = 0.5 * (lo + hi)
        if (_win64(l, r, m) < THRESH) == rising:
            lo = m
        else:
            hi = m
    return 0.5 * (lo + hi)


def _refine_flip_fp32(l32, r32, b64, rising):
    """Exact fp32 x where the reference's jax-fp32 predicate win(x)>1e-3 flips.
    Returns the smallest fp32 x at which the predicate equals its right-side
    state. Falls back to the float64 bisection value if jax is unavailable."""
    try:
        import jax
        import jax.numpy as jnp

        cpu = jax.devices("cpu")[0]
        lo = np.float32(b64 - 5e-5)
        hi = np.float32(b64 + 5e-5)
        xs = np.arange(lo.view(np.int32), hi.view(np.int32) + 1,
                       dtype=np.int32).view(np.float32)
        with jax.default_device(cpu):
            win = np.asarray(
                jax.nn.sigmoid(jnp.asarray(xs) - np.float32(l32))
                * jax.nn.sigmoid(-(jnp.asarray(xs) - np.float32(r32)))
            )
        pred = win > np.float32(THRESH)
        state = pred if rising else ~pred
        if not state.any() or state.all():
            return np.float32(b64)
        k = int(np.argmax(state))
        if not state[k:].all():
            return np.float32(b64)
        return xs[k]
    except Exception:
        return np.float32(b64)


_GEOM = None


def _geometry():
    global _GEOM
    if _GEOM is not None:
        return _GEOM
    means, std, mid = _partition_geom()
    ml = mid[:-1].astype(np.float64)
    mr = mid[1:].astype(np.float64)
    Lb = np.zeros(NW, np.float32)   # window-on lower bound (exact fp32 flip)
    Rb = np.zeros(NW, np.float32)   # window-off upper bound
    for w in range(NW):
        c = 0.5 * (ml[w] + mr[w])
        l64 = _bisect64(ml[w], mr[w], ml[w] - 30, c, rising=True)
        r64 = _bisect64(ml[w], mr[w], c, mr[w] + 30, rising=False)
        Lb[w] = _refine_flip_fp32(mid[w], mid[w + 1], l64, rising=True)
        Rb[w] = _refine_flip_fp32(mid[w], mid[w + 1], r64, rising=False)
    _GEOM = (means, std, mid, Lb, Rb)
    return _GEOM


def _active_windows(core):
    means, std, mid, Lb, Rb = _geometry()
    base = DOM0 + core * DW
    return [w for w in range(NW) if (Rb[w] > base) and (Lb[w] < base + DW)]


# ---------------- bass program (built once per S, SPMD across 8 cores) --------
_PROGS = {}


def _build_program(S):
    if S in _PROGS:
        return _PROGS[S]
    from concourse import bacc, mybir, tile

    f32 = mybir.dt.float32
    f16 = mybir.dt.float16
    Act = mybir.ActivationFunctionType
    Op = mybir.AluOpType

    assert S % NCHO == 0
    CHO = S // NCHO

    nc = bacc.Bacc(None, target_bir_lowering=False)

    tw_in = nc.declare_dram_parameter("tw", [P, W2C + S], f16, isOutput=False)
    c0_in = nc.declare_dram_parameter("cpack0", [P, 2 * NG], f32, isOutput=False)
    ca_in = nc.declare_dram_parameter("cpacka", [P, CWA], f32, isOutput=False)
    cb_in = nc.declare_dram_parameter("cpackb", [PB, CWB], f32, isOutput=False)
    y_out = nc.declare_dram_parameter("y_out", [P, S], f16, isOutput=True)

    with tile.TileContext(nc) as tc:
        with (
            tc.tile_pool(name="const", bufs=1) as cpool,
            tc.tile_pool(name="work", bufs=2) as wpool,
            tc.tile_pool(name="pts", bufs=4) as ppool,
            tc.tile_pool(name="psum", bufs=1, space="PSUM") as psum,
        ):
            # ---- prefetch the Tanh ACT table behind the const DMAs ----
            dmy = cpool.tile([1, 4], f32, tag="dmy")
            nc.vector.memset(dmy[:], 0.0)
            dmy2 = cpool.tile([1, 4], f32, tag="dmy2")
            nc.scalar.activation(out=dmy2[:], in_=dmy[:], func=Act.Tanh)

            # ---- input DMAs, descriptor-count aware: the tiny const packs
            # issue FIRST so their descriptors sit at every queue head; the
            # big f16 stream (w2 + points) is ONE dma behind them ----
            # all three input DMAs on the SYNC ring so queue order is
            # strictly cpacka -> tw -> cpackb (parallel rings would
            # interleave descriptors and delay the h1-critical pack)
            cp0 = cpool.tile([P, 2 * NG], f32, tag="c_cp0")
            nc.sync.dma_start(out=cp0[:], in_=c0_in[:])
            cpk = cpool.tile([P, CWA], f32, tag="c_cpk")
            nc.sync.dma_start(out=cpk[:], in_=ca_in[:])
            tw = cpool.tile([P, W2C + S], f16, tag="c_tw")
            nc.sync.dma_start(out=tw[:, 0:W2C], in_=tw_in[:, 0:W2C])
            cpb = cpool.tile([PB, CWB], f32, tag="c_cpb")
            nc.sync.dma_start(out=cpb[:], in_=cb_in[:])
            nc.sync.dma_start(out=tw[:, W2C:W2C + S], in_=tw_in[:, W2C:W2C + S])
            # knot index row on gpsimd (free ring)
            krow = cpool.tile([P, NKP], f32, tag="c_krf")
            nc.gpsimd.iota(krow[:], pattern=[[1, NKP]], channel_multiplier=0,
                           allow_small_or_imprecise_dtypes=True)

            # ---- p-state warm-up: keep PE/DVE busy during the dead window ----
            wz = cpool.tile([P, 640], f16, tag="warmz")
            nc.vector.memset(wz[:], 0.0)
            wz2 = cpool.tile([P, 512], f16, tag="warmz2")
            nc.vector.tensor_scalar(out=wz2[:], in0=wz[:, 0:512], scalar1=1.0,
                                    scalar2=None, op0=Op.mult)
            wps = psum.tile([P, 512], f32, tag="wps")
            for _ in range(3):
                nc.tensor.matmul(out=wps[:], lhsT=wz[:, 0:P], rhs=wz[:, P:P + 512],
                                 start=True, stop=True)

            sc1h = cp0[:, 0:NG]
            bi1b = cp0[:, NG:2 * NG]
            b2c = cpk[:, O_B2:O_B2 + NG]
            w3c = cpk[:, O_W3:O_W3 + NSLOT * NG]
            xl = cpk[:, O_XL:O_XL + 1]
            xr = cpk[:, O_XR:O_XR + 1]
            s12 = cpb[0:PB, O_S12:O_S12 + 1]
            b12 = cpb[0:PB, O_B12:O_B12 + 1]
            wm = cpb[0:NSLOT, O_WM:O_WM + NK]
            wm32 = cpb[32:PB, O_WM:O_WM + NK]
            b3c = cpb[0:NSLOT, O_B3:O_B3 + 1]
            onc = cpb[0:NSLOT, O_ON:O_ON + 1]
            w2 = tw[:, 0:W2C]
            tp = tw[:, W2C:W2C + S]

            # fp16 copies of the tiny matmul operands
            w3f = cpool.tile([P, NSLOT * NG], f16, tag="c_w3f")
            nc.vector.tensor_copy(out=w3f[:], in_=w3c)
            on16 = cpool.tile([NSLOT, 1], f16, tag="c_on16")
            nc.vector.tensor_copy(out=on16[:], in_=onc)

            # ---- phase B: knot values (single chunk, all-Tanh, fp16 mm) ----
            h1s, h2ps = [], []
            for g in range(NG):
                h1 = wpool.tile([P, NK], f16, tag=f"h1_{g}")
                nc.scalar.activation(out=h1[:], in_=krow[:, :NK], func=Act.Tanh,
                                     bias=bi1b[:, g:g + 1], scale=sc1h[:, g:g + 1])
                h2p = psum.tile([P, NK], f32, tag=f"h2p_{g}")
                nc.tensor.matmul(out=h2p[:], lhsT=w2[:, g * P:(g + 1) * P],
                                 rhs=h1[:], start=True, stop=True)
                h1s.append(h1)
                h2ps.append(h2p)
            # window tanhs issued between h1 and h2 acts: they have no matmul
            # dependency, so ACT stays busy while the h2p matmuls run.
            # rows 0:12 of t12 are tanh(+(x-l)/2), rows 32:44 tanh(-(x-r)/2)
            # (second bank at partition 32: engine partition windows must be
            # 32-aligned)
            t12 = wpool.tile([32 + NSLOT, NK], f32, tag="t12")
            nc.scalar.activation(out=t12[:], in_=krow[0:32 + NSLOT, :NK],
                                 func=Act.Tanh, scale=s12, bias=b12)
            h2s = []
            for g in range(NG):
                h2 = wpool.tile([P, NK], f16, tag=f"h2_{g}")
                nc.scalar.activation(out=h2[:], in_=h2ps[g][:], func=Act.Tanh,
                                     bias=b2c[:, g:g + 1], scale=1.0)
                h2s.append(h2)
            pre = psum.tile([NSLOT, NK], f32, tag="pre")
            for g in range(NG):
                nc.tensor.matmul(out=pre[:], lhsT=w3f[:, g * NSLOT:(g + 1) * NSLOT],
                                 rhs=h2s[g][:], start=(g == 0), stop=(g == NG - 1))
            wbm = wpool.tile([NSLOT, NK], f32, tag="wbm")
            nc.vector.scalar_tensor_tensor(out=wbm[:], in0=t12[32:32 + NSLOT, :],
                                           scalar=1.0, in1=wm32, op0=Op.add,
                                           op1=Op.mult)
            win = wpool.tile([NSLOT, NK], f32, tag="win")
            nc.vector.scalar_tensor_tensor(out=win[:], in0=t12[0:NSLOT, :],
                                           scalar=1.0, in1=wbm[:], op0=Op.add,
                                           op1=Op.mult)
            term = wpool.tile([NSLOT, NK], f16, tag="term")
            nc.vector.scalar_tensor_tensor(out=term[:], in0=pre[:], scalar=b3c,
                                           in1=win[:], op0=Op.add, op1=Op.mult)

            # ---- phase C: partition-major knot sums + cell records ----
            vlo = psum.tile([P, 1], f32, tag="vlo")
            nc.tensor.matmul(out=vlo[:], lhsT=term[:, 0:P], rhs=on16[:],
                             start=True, stop=True)
            vhi = psum.tile([P, 1], f32, tag="vhi")
            nc.tensor.matmul(out=vhi[:], lhsT=term[:, 1:P + 1], rhs=on16[:],
                             start=True, stop=True)
            thb = wpool.tile([P, 2], f32, tag="thb")
            nc.scalar.activation(out=thb[:], in_=cpk[:, O_XL:O_XL + 2],
                                 func=Act.Tanh)
            ulos = wpool.tile([P, 1], f32, tag="ulos")
            nc.vector.tensor_mul(out=ulos[:], in0=vlo[:], in1=thb[:, 0:1])
            dvs = wpool.tile([P, 1], f32, tag="dvs")
            nc.vector.scalar_tensor_tensor(out=dvs[:], in0=vhi[:],
                                           scalar=thb[:, 1:2], in1=ulos[:],
                                           op0=Op.mult, op1=Op.subtract)

            # ---- phase D: y = t*dv + ulo, fp16 in/out, one fused op/chunk ----
            oeng = [nc.sync, nc.scalar]
            for ch in range(NCHO):
                sl = slice(ch * CHO, (ch + 1) * CHO)
                y = ppool.tile([P, CHO], f16, tag="y")
                nc.vector.tensor_scalar(out=y[:], in0=tp[:, sl], scalar1=dvs[:],
                                        scalar2=ulos[:], op0=Op.mult, op1=Op.add)
                oeng[ch % 2].dma_start(out=y_out[:, sl], in_=y[:])

    nc.compile()
    _PROGS[S] = nc
    return nc


# ---------------- host-side input prep ----------------------------------------
def _fold_weights(core, W1, b1, W2, b2, W3, b3):
    means, std, mid, Lb, Rb = _geometry()
    base = DOM0 + core * DW
    act = _active_windows(core)
    assert len(act) <= NSLOT, f"core {core}: {len(act)} active windows"
    cpacka = np.zeros((P, CWA), np.float32)
    cpackb = np.zeros((PB, CWB), np.float32)
    w2pack = np.zeros((P, W2C), np.float16)
    sc1h = np.zeros((P, NG), np.float64)
    bi1b = np.zeros((P, NG), np.float64)
    for slot, w in enumerate(act):
        g, s = divmod(slot, 4)
        rows = slice(32 * s, 32 * s + 32)
        w1r = W1[w, 0, :].astype(np.float64)
        sc1h[rows, g] = w1r / std[w] * H
        bi1b[rows, g] = b1[w] + w1r * (base - means[w]) / std[w]
        w2pack[rows, g * P + 32 * s:g * P + 32 * s + 32] = W2[w].astype(np.float16)
        cpacka[rows, O_W3 + g * NSLOT + slot] = W3[w, :, 0]
        cpacka[rows, O_B2 + g] = b2[w]
        cpackb[slot, O_B3] = b3[w, 0]
        cpackb[slot, O_B12] = np.float32((base - np.float64(mid[w])) / 2.0)
        cpackb[32 + slot, O_B12] = np.float32((np.float64(mid[w + 1]) - base) / 2.0)
    cpacka[:, O_SC1:O_SC1 + NG] = sc1h.astype(np.float32)
    cpacka[:, O_BI1:O_BI1 + NG] = bi1b.astype(np.float32)
    cpackb[0:NSLOT, O_S12] = H / 2
    cpackb[32:PB, O_S12] = -H / 2
    # cell-left/right x per partition (exact in fp32)
    cpacka[:, O_XL] = (np.float64(base)
                       + np.arange(P, dtype=np.float64) * H).astype(np.float32)
    cpacka[:, O_XR] = (np.float64(base)
                       + np.arange(1, P + 1, dtype=np.float64) * H).astype(np.float32)
    # window mask at knots (0.25 factor of the tanh-sigmoid identity folded in)
    kx = (np.float64(base) + np.arange(NK, dtype=np.float64) * H).astype(np.float32)
    for slot, w in enumerate(act):
        lbv = np.nextafter(Lb[w], -np.inf)
        mrow = 0.25 * ((kx > lbv) & (kx < Rb[w]))
        cpackb[slot, O_WM:O_WM + NK] = mrow
        cpackb[32 + slot, O_WM:O_WM + NK] = mrow   # copy at partition base 32
    cpackb[0:NSLOT, O_ON] = 1.0
    # inactive slots (and the 12:32 alignment gap): park the window far away
    for slot in range(len(act), NSLOT):
        cpackb[slot, O_B12] = -1e4
        cpackb[32 + slot, O_B12] = -1e4
    cpackb[NSLOT:32, O_B12] = -1e4
    return cpacka, cpackb, w2pack


def _prep_in_maps(inputs, S):
    x = np.asarray(inputs["x"], np.float32)
    W1 = np.asarray(inputs["W1"], np.float32)
    b1 = np.asarray(inputs["b1"], np.float32)
    W2 = np.asarray(inputs["W2"], np.float32)
    b2 = np.asarray(inputs["b2"], np.float32)
    W3 = np.asarray(inputs["W3"], np.float32)
    b3 = np.asarray(inputs["b3"], np.float32)

    ncell = NCORES * C_LOC
    cglob = np.minimum((x.astype(np.float64) * (1.0 / H)).astype(np.int64),
                       ncell - 1)
    order = np.argsort(cglob, kind="stable")
    cs = cglob[order]
    cnt = np.bincount(cglob, minlength=ncell)
    maxcnt = int(cnt.max())
    if maxcnt > S:
        raise OverflowError(maxcnt)
    starts = np.concatenate(([0], np.cumsum(cnt)))
    rank = np.arange(len(x)) - starts[cs]           # rank within own cell
    slot = cs * S + rank                            # global padded slot index

    # t = (x - cell_left)/h in [0,1), sent as fp16
    cellxg = (cglob.astype(np.float64) * H).astype(np.float32)   # exact fp32
    tval = ((x - cellxg) * np.float32(INVH)).astype(np.float16)

    in_maps = []
    for core in range(NCORES):
        tw = np.zeros((P, W2C + S), np.float16)
        msk = (cs >= core * C_LOC) & (cs < (core + 1) * C_LOC)
        loc = slot[msk] - core * C_LOC * S
        tw[loc // S, W2C + loc % S] = tval[order[msk]]
        cpacka, cpackb, w2pack = _fold_weights(core, W1, b1, W2, b2, W3, b3)
        tw[:, 0:W2C] = w2pack
        in_maps.append({
            "tw": tw,
            "cpack0": np.ascontiguousarray(cpacka[:, O_SC1:O_SC1 + 2 * NG]),
            "cpacka": cpacka,
            "cpackb": cpackb,
        })
    return in_maps, order, slot


def _unpack(results, order, slot, n_total):
    allys = np.concatenate([r["y_out"].reshape(-1) for r in results])
    out = np.empty(n_total, np.float32)
    out[order] = allys[slot].astype(np.float32)
    return out


def kernel(**inputs) -> np.ndarray:
    from concourse.bass_utils import run_bass_kernel_spmd

    S = S_DEFAULT
    while True:
        try:
            in_maps, order, slot = _prep_in_maps(inputs, S)
            break
        except OverflowError as e:
            S = ((int(e.args[0]) + 23) // 16) * 16   # headroom, multiple of 16
    nc = _build_program(S)
    res = run_bass_kernel_spmd(nc, in_maps, list(range(NCORES)))
    return _unpack(res.results, order, slot, len(np.asarray(inputs["x"])))
